# revision 13
# baseline (speedup 1.0000x reference)
"""Trainium2 Bass kernel for a 4-layer dense MLP (H=8192), batch=1.

Tensor-parallel over 8 NeuronCores. Structure (per core):

  - Layer 1 (10x8192, replicated) computes straight into the [128, 2, 32]
    chunked activation layout: 64 matmuls with [11, 128] stationary
    slices of an augmented (W_in | bias0) matrix and moving x_aug
    ([x; s; 1]), sigmoid straight to fp8. No DRAM bounce.

  - Hidden layers 2-4 are column-sharded (core c owns 1024 columns),
    each computed as two 512-col output halves A/B so the AllGather of
    half A overlaps half B's compute; the next layer's contraction is
    ordered gathered-A-first (host-side weight-row permutations).

  - AllGather outputs ([8 ranks x 512] f16 in DRAM) are unpacked with a
    contiguous [8, 512] SBUF load + 4 PE-transpose ops (identity rhs)
    instead of a 128-line scatter DMA — the scatter cost ~10us per
    boundary on the critical path.

  - Weight precision (the problem is HBM-bound; fp8 halves the 48
    MiB/core fp16 stream): L2, L3 fp8 e4m3 (x 2^13) with DoubleRow perf
    mode (2x PE ingestion; their input activations quantize to e4m3);
    L4 fp8 e3m4 (x 2^9, extra mantissa bit) with fp16 activations.
    Descales fold into the sigmoid's scale argument. Host-sim error vs
    the f32 reference: ~4.7e-3 (max-abs / max-abs-ref).

  - Output layer (8192x8) row-sharded: activations transposed into
    [128, 8] via PE transposes (no DRAM bounce), partial [8] per core,
    host sums. A dummy AllGather (shaped like the real ones so the mesh
    algo setup is warmed too) absorbs the one-time ncfw rendezvous.

Weights stream as 1 MiB DMAs (8 KiB per partition line) into
[128, 16, 512] SBUF tiles, 16 in flight: L3+L4 fully buffered during
the collective phase, so the post-barrier chain is pure PE + AG.
"""

import numpy as np

H = 8192
D = 10  # input layer size (4 + 6)
DA = D + 1  # augmented with the bias row
OUT = 8
NCORES = 8
SH = H // NCORES  # 1024 columns per core
HF = 512  # half-width
KC = 64  # contraction chunks of 128 rows
GC = 16  # chunks per DMA group (1 MiB per DMA)
G = KC // GC  # 4 groups per output half
WBUFS = 16  # in-flight weight DMA buffers (16 MiB SBUF)
S_DR = float(2**13)  # e4m3 weight scale (|W|max*2^13 ~ 157 < 240)
S_E3 = float(2**9)  # e3m4 weight scale (|W|max*2^9 ~ 9.8 < 15.5)

LAST_RESULTS = None
_CACHE = {}


def _build_nc():
    import concourse.bacc as bacc
    import concourse.mybir as mybir
    import concourse.tile as tile

    f16 = mybir.dt.float16
    f32 = mybir.dt.float32
    f8e4 = mybir.dt.float8e4
    f8e3 = mybir.dt.float8e3
    SIG = mybir.ActivationFunctionType.Sigmoid
    DR = mybir.MatmulPerfMode.DoubleRow
    RG = [list(range(NCORES))]

    nc = bacc.Bacc(
        "TRN2", target_bir_lowering=False, debug=False, num_devices=NCORES
    )

    x_d = nc.dram_tensor("x_aug", [DA, 1], f16, kind="ExternalInput")
    win_d = nc.dram_tensor("w_in", [DA, H], f16, kind="ExternalInput")
    w2_d = nc.dram_tensor("w_l2", [2, G, 128, GC, HF], f8e4, kind="ExternalInput")
    w3_d = nc.dram_tensor("w_l3", [2, G, 128, GC, HF], f8e4, kind="ExternalInput")
    w4_d = nc.dram_tensor("w_l4", [2, G, 128, GC, HF], f8e4, kind="ExternalInput")
    wout_d = nc.dram_tensor("w_out", [128, 8 * OUT], f16, kind="ExternalInput")
    bias_d = nc.dram_tensor("bias", [1, 3 * SH], f16, kind="ExternalInput")
    id_d = nc.dram_tensor("ident", [8, 8], f16, kind="ExternalInput")
    out_d = nc.dram_tensor("out_partial", [1, OUT], f32, kind="ExternalOutput")

    with tile.TileContext(nc) as tc:
        with (
            tc.tile_pool(name="const", bufs=1) as cp,
            tc.tile_pool(name="wpool", bufs=WBUFS) as wp,
            tc.tile_pool(name="apool", bufs=2) as ap,
            tc.tile_pool(name="pspool", bufs=2, space="PSUM") as pp,
            tc.tile_pool(name="dpool", bufs=2, space="DRAM") as dp,
        ):
            # Dummy collectives on the REAL cc buffer pairs (both
            # channels): absorbs the one-time ncfw rendezvous AND the
            # per-channel mesh algo setup concurrently with layer-1
            # compute + weight prefetch. Content is garbage; the real
            # AllGathers fully overwrite the outputs later.
            cc_in_t = [None, None]
            cc_out_t = [None, None]
            for h in range(2):
                cc_in_t[h] = dp.tile(
                    [1, HF], f16, tag=f"ccin{h}", bufs=1, name="cc_in"
                )
                cc_out_t[h] = dp.tile(
                    [8, HF], f16, tag=f"ccout{h}", bufs=1, name="cc_out"
                )
                nc.gpsimd.collective_compute(
                    "AllGather",
                    mybir.AluOpType.bypass,
                    replica_groups=RG,
                    ins=[cc_in_t[h].opt()],
                    outs=[cc_out_t[h].opt()],
                )

            one_sb = cp.tile([1, 1], f16)
            nc.gpsimd.memset(one_sb[:], 1.0)

            x_sb = cp.tile([DA, 1], f16)
            nc.scalar.dma_start(x_sb[:], x_d[:])
            win_sb = cp.tile([DA, H], f16)
            nc.scalar.dma_start(win_sb[:], win_d[:])
            bias_sb = cp.tile([1, 3 * SH], f16)
            nc.scalar.dma_start(bias_sb[:], bias_d[:])
            wout_sb = cp.tile([128, 8 * OUT], f16)
            nc.scalar.dma_start(wout_sb[:], wout_d[:])
            ident_sb = cp.tile([8, 8], f16)
            nc.scalar.dma_start(ident_sb[:], id_d[:])

            # ---- Layer 1, replicated: straight into the [128, 2, 32]
            # layout (a8_sb[p, i, c] = a1[(i*32+c)*128 + p]); the (2, 32)
            # split gives DoubleRow lhsT pairs a 32 B pair stride. ----
            a8_sb = ap.tile([128, 2, KC // 2], f8e4, tag="a8")
            for j8 in range(8):
                hi, c0 = j8 // 4, (8 * j8) % 32
                ps1 = pp.tile([128, 1, 8], f32, tag="psL1", bufs=1)
                for jj in range(8):
                    j = 8 * j8 + jj
                    nc.tensor.matmul(
                        ps1[:, 0:1, jj : jj + 1],
                        win_sb[:, 128 * j : 128 * j + 128],
                        x_sb[:],
                        start=True,
                        stop=True,
                    )
                nc.scalar.activation(
                    a8_sb[:, hi : hi + 1, c0 : c0 + 8], ps1[:], SIG
                )

            # ---- helpers ----
            def emit_gather(act_h, hf):
                """AllGather one 512-col output half; returns cc_out."""
                cc_in, cc_out = cc_in_t[hf], cc_out_t[hf]
                nc.scalar.dma_start(cc_in[:], act_h[:])
                nc.gpsimd.collective_compute(
                    "AllGather",
                    mybir.AluOpType.bypass,
                    replica_groups=RG,
                    ins=[cc_in.opt()],
                    outs=[cc_out.opt()],
                )
                return cc_out

            def emit_unpack(cc_out, a_dst, hf_in, dr):
                """[8, 512] gathered half -> 4 PE transposes -> a_dst.
                Column q=8j+r of the transpose holds rank r's cols
                [128j, 128j+128) of this half."""
                g8 = ap.tile([8, HF], f16, tag="g8", name="g8")
                nc.scalar.dma_start(g8[:], cc_out[:])
                if dr:
                    psT = pp.tile([128, 2, 16], f16, tag="psT3", bufs=1)
                    for j in range(4):
                        nc.tensor.matmul(
                            psT[:, j // 2 : j // 2 + 1, 8 * (j % 2) : 8 * (j % 2) + 8],
                            g8[:, 128 * j : 128 * j + 128],
                            ident_sb[:],
                            is_transpose=True,
                            start=True,
                            stop=True,
                        )
                    for i in range(2):
                        nc.vector.tensor_copy(
                            a_dst[:, i : i + 1, 16 * hf_in : 16 * hf_in + 16],
                            psT[:, i : i + 1, :],
                        )
                else:
                    psT = pp.tile([128, 32], f16, tag="psT4", bufs=1)
                    for j in range(4):
                        nc.tensor.matmul(
                            psT[:, 8 * j : 8 * j + 8],
                            g8[:, 128 * j : 128 * j + 128],
                            ident_sb[:],
                            is_transpose=True,
                            start=True,
                            stop=True,
                        )
                    nc.vector.tensor_copy(
                        a_dst[:, 32 * hf_in : 32 * hf_in + 32], psT[:]
                    )

            def emit_hidden(
                w_d, pm, descale, a_in, bias_off, inject_b=None, after_half=None
            ):
                """One hidden layer: 2 output halves x 4 weight groups.
                inject_b() is called before group 2 of half 0 — the spot
                where the previous boundary's B-half unpack goes (its
                AG has landed by then; groups 0-1 touch only A data).
                after_half(hf, act) runs right after each half's sigmoid
                so gather DMAs don't queue behind the other half."""
                outs = []
                for hf in range(2):
                    ps = pp.tile([1, HF], f32, tag="psH", bufs=2, name="ps")
                    for g in range(G):
                        if inject_b is not None and hf == 0 and g == 2:
                            inject_b()
                        wt = wp.tile(
                            [128, GC, HF],
                            f8e4 if pm is DR else f8e3,
                            tag="w",
                            name="wt",
                        )
                        nc.sync.dma_start(wt[:], w_d[hf, g])
                        if pm is DR:
                            for c in range(GC // 2):
                                k = g * GC + 2 * c
                                nc.tensor.matmul(
                                    ps[:],
                                    a_in[:, :, k // 2 : k // 2 + 1],
                                    wt[:, 2 * c : 2 * c + 2, :],
                                    start=(k == 0),
                                    stop=False,
                                    perf_mode=DR,
                                )
                        else:
                            for c in range(GC):
                                k = g * GC + c
                                nc.tensor.matmul(
                                    ps[:],
                                    a_in[:, k : k + 1],
                                    wt[:, c : c + 1, :],
                                    start=(k == 0),
                                    stop=False,
                                )
                    nc.tensor.matmul(
                        ps[:],
                        one_sb[:],
                        bias_sb[:, bias_off + hf * HF : bias_off + hf * HF + HF],
                        start=False,
                        stop=True,
                    )
                    act_h = ap.tile([1, HF], f16, tag=f"act{hf}", name="act_h")
                    nc.scalar.activation(act_h[:], ps[:], SIG, scale=descale)
                    if after_half is not None:
                        after_half(hf, act_h)
                    outs.append(act_h)
                return outs

            # ---- Layer 2 (DoubleRow e4m3) ----
            ccs = [None, None]

            def gather_cb(hf, a):
                ccs[hf] = emit_gather(a, hf)

            emit_hidden(w2_d, DR, 1.0 / S_DR, a8_sb, 0, after_half=gather_cb)
            cc_a, cc_b = ccs

            # ---- Layer 3 (DoubleRow e4m3): input a3 [128, 2, 32] fp8 ----
            a3_sb = ap.tile([128, 2, KC // 2], f8e4, tag="a3")
            emit_unpack(cc_a, a3_sb, 0, dr=True)
            ccb = cc_b

            def inject3(cc=ccb):
                emit_unpack(cc, a3_sb, 1, dr=True)

            emit_hidden(
                w3_d, DR, 1.0 / S_DR, a3_sb, SH,
                inject_b=inject3, after_half=gather_cb,
            )
            cc_a, cc_b = ccs

            # ---- Layer 4 (DoubleRow e4m3): input a4 [128, 2, 32] fp8 ----
            a4_sb = ap.tile([128, 2, KC // 2], f8e4, tag="a4")
            emit_unpack(cc_a, a4_sb, 0, dr=True)
            ccb2 = cc_b

            def inject4(cc=ccb2):
                emit_unpack(cc, a4_sb, 1, dr=True)

            act = emit_hidden(w4_d, DR, 1.0 / S_DR, a4_sb, 2 * SH, inject_b=inject4)

            # ---- Output layer: transpose acts to [128, 8], row-sharded
            # partial [8] per core (a2_sb[p, t] = act_local[128t + p]) ----
            psOT = pp.tile([128, 8, 2], f16, tag="psOT", bufs=1)
            for t in range(8):
                hf, off = t // 4, 128 * (t % 4)
                nc.tensor.matmul(
                    psOT[:, t : t + 1, 0:1],
                    act[hf][:, off : off + 128],
                    one_sb[:],
                    is_transpose=True,
                    start=True,
                    stop=True,
                )
            a2_sb = ap.tile([128, 8, 2], f16, tag="a2")
            nc.vector.tensor_copy(a2_sb[:], psOT[:])
            pso = pp.tile([1, OUT], f32, tag="psO", bufs=1)
            for t in range(8):
                nc.tensor.matmul(
                    pso[:],
                    a2_sb[:, t : t + 1, 0:1],
                    wout_sb[:, t * OUT : (t + 1) * OUT],
                    start=(t == 0),
                    stop=(t == 7),
                )
            res_sb = ap.tile([1, OUT], f32, tag="res")
            nc.vector.tensor_copy(res_sb[:], pso[:])
            nc.scalar.dma_start(out_d[:], res_sb[:])

    nc.compile()
    return nc


def _pack_layer(wcol_q, perm):
    """[8192, 1024] quantized core shard -> [2 halves, G, 128, GC, HF],
    rows permuted so weight block b, partition p holds row perm[b, p]."""
    wperm = wcol_q[perm]  # [KC, 128, 1024]
    grp = wperm.reshape(G, GC, 128, 2 * HF).transpose(0, 2, 1, 3)  # [G,128,GC,1024]
    return np.stack([grp[..., :HF], grp[..., HF:]])  # [2, G, 128, GC, HF]


def _prep_inputs(x, s, W_in, W_hh, W_out, b):
    """Shard + quantize + lay out the inputs for each of the 8 cores."""
    import ml_dtypes

    f16 = np.float16
    e4 = ml_dtypes.float8_e4m3
    e3 = ml_dtypes.float8_e3m4

    x_aug = np.concatenate(
        [np.asarray(x), np.asarray(s), np.ones(1, np.float32)]
    ).astype(f16)
    x_aug = np.ascontiguousarray(x_aug.reshape(DA, 1))
    b32 = np.asarray(b, np.float32)  # [5, 8192] (b[4] unused)
    win_aug = np.ascontiguousarray(
        np.concatenate([np.asarray(W_in), b32[0:1]], axis=0).astype(f16)
    )  # [11, 8192]
    Whh = np.asarray(W_hh, np.float32)  # [3, 8192, 8192]
    Wout32 = np.asarray(W_out, np.float32)  # [8192, 8]

    # weight block b (or chunk k), partition p -> global activation row.
    bb = np.arange(KC)[:, None]
    p = np.arange(128)[None, :]
    # L2 (DR): block b pairs with a8 col q=(b%2)*32+b//2 = rows q*128+p.
    perm_l2 = (((bb % 2) * 32 + bb // 2) * 128) + p
    # L3 (DR): pair c=b//2 (c<16: A half), i=b%2; in-half chunk
    # q=16i+(c%16); row = (q%8)*1024 + (c//16)*512 + (q//8)*128 + p.
    c_, i_ = bb // 2, bb % 2
    q_ = 16 * i_ + (c_ % 16)
    perm_l3 = (q_ % 8) * 1024 + (c_ // 16) * 512 + (q_ // 8) * 128 + p
    perm_l4 = perm_l3  # L4 (DR): same unpack scheme as L3

    bias_rows = np.concatenate(
        [b32[1] * S_DR, b32[2] * S_DR, b32[3] * S_DR]
    ).astype(f16)  # [3*8192], host-scaled (zeros in this problem)

    ident = np.eye(8, dtype=f16)

    in_maps = []
    for c in range(NCORES):
        cs, ce = c * SH, (c + 1) * SH
        w2 = _pack_layer((Whh[0][:, cs:ce] * S_DR).astype(e4), perm_l2)
        w3 = _pack_layer((Whh[1][:, cs:ce] * S_DR).astype(e4), perm_l3)
        w4 = _pack_layer((Whh[2][:, cs:ce] * S_DR).astype(e4), perm_l4)
        bias_c = np.concatenate(
            [bias_rows[li * H + cs : li * H + ce] for li in range(3)]
        ).reshape(1, 3 * SH)
        # out layer: a2_sb[p, t] = act_local[128t + p]
        wout_c = np.ascontiguousarray(
            Wout32[cs:ce].reshape(8, 128, OUT).transpose(1, 0, 2)
            .reshape(128, 8 * OUT).astype(f16)
        )
        in_maps.append(
            {
                "x_aug": x_aug,
                "w_in": win_aug,
                "w_l2": np.ascontiguousarray(w2),
                "w_l3": np.ascontiguousarray(w3),
                "w_l4": np.ascontiguousarray(w4),
                "w_out": wout_c,
                "bias": np.ascontiguousarray(bias_c),
                "ident": ident,
            }
        )
    return in_maps


def kernel(**inputs):
    global LAST_RESULTS
    import os

    from concourse import bass_utils

    if "nc" not in _CACHE:
        _CACHE["nc"] = _build_nc()
    nc = _CACHE["nc"]

    in_maps = _prep_inputs(**inputs)
    trace = bool(int(os.environ.get("BASS_TRACE_KERNEL", "0")))
    res = bass_utils.run_bass_kernel_spmd(
        nc, in_maps, core_ids=list(range(NCORES)), trace=trace
    )
    LAST_RESULTS = res
    partials = np.stack([r["out_partial"][0] for r in res.results])  # [8, 8]
    return partials.sum(axis=0).astype(np.float32)


# revision 14
# speedup vs baseline: 1.0502x; 1.0502x over previous
"""Trainium2 Bass kernel for a 4-layer dense MLP (H=8192), batch=1.

Tensor-parallel over 8 NeuronCores. Structure (per core):

  - Layer 1 (10x8192, replicated) computes straight into the [128, 2, 32]
    chunked activation layout: 64 matmuls with [11, 128] stationary
    slices of an augmented (W_in | bias0) matrix and moving x_aug
    ([x; s; 1]), sigmoid straight to fp8. No DRAM bounce.

  - Hidden layers 2-4 are column-sharded (core c owns 1024 columns),
    each computed as two 512-col output halves A/B so the AllGather of
    half A overlaps half B's compute; the next layer's contraction is
    ordered gathered-A-first (host-side weight-row permutations).

  - AllGather outputs ([8 ranks x 512] f16 in DRAM) are unpacked with a
    contiguous [8, 512] SBUF load + 4 PE-transpose ops (identity rhs)
    instead of a 128-line scatter DMA — the scatter cost ~10us per
    boundary on the critical path.

  - Weight precision (the problem is HBM-bound; fp8 halves the 48
    MiB/core fp16 stream): L2, L3 fp8 e4m3 (x 2^13) with DoubleRow perf
    mode (2x PE ingestion; their input activations quantize to e4m3);
    L4 fp8 e3m4 (x 2^9, extra mantissa bit) with fp16 activations.
    Descales fold into the sigmoid's scale argument. Host-sim error vs
    the f32 reference: ~4.7e-3 (max-abs / max-abs-ref).

  - Output layer (8192x8) row-sharded: activations transposed into
    [128, 8] via PE transposes (no DRAM bounce), partial [8] per core,
    host sums. A dummy AllGather (shaped like the real ones so the mesh
    algo setup is warmed too) absorbs the one-time ncfw rendezvous.

Weights stream as 1 MiB DMAs (8 KiB per partition line) into
[128, 16, 512] SBUF tiles, 16 in flight: L3+L4 fully buffered during
the collective phase, so the post-barrier chain is pure PE + AG.
"""

import numpy as np

H = 8192
D = 10  # input layer size (4 + 6)
DA = D + 1  # augmented with the bias row
OUT = 8
NCORES = 8
SH = H // NCORES  # 1024 columns per core
HF = 512  # half-width
KC = 64  # contraction chunks of 128 rows
GC = 16  # chunks per DMA group (1 MiB per DMA)
G = KC // GC  # 4 groups per output half
WBUFS = 16  # in-flight weight DMA buffers (16 MiB SBUF)
S_DR = float(2**13)  # e4m3 weight scale (|W|max*2^13 ~ 157 < 240)
S_E3 = float(2**9)  # e3m4 weight scale (|W|max*2^9 ~ 9.8 < 15.5)

LAST_RESULTS = None
_CACHE = {}


def _build_nc():
    import concourse.bacc as bacc
    import concourse.mybir as mybir
    import concourse.tile as tile

    f16 = mybir.dt.float16
    f32 = mybir.dt.float32
    f8e4 = mybir.dt.float8e4
    f8e3 = mybir.dt.float8e3
    SIG = mybir.ActivationFunctionType.Sigmoid
    DR = mybir.MatmulPerfMode.DoubleRow
    RG = [list(range(NCORES))]

    nc = bacc.Bacc(
        "TRN2", target_bir_lowering=False, debug=False, num_devices=NCORES
    )

    x_d = nc.dram_tensor("x_aug", [DA, 1], f16, kind="ExternalInput")
    win_d = nc.dram_tensor("w_in", [DA, H], f16, kind="ExternalInput")
    w2_d = nc.dram_tensor("w_l2", [2, G, 128, GC, HF], f8e4, kind="ExternalInput")
    w3_d = nc.dram_tensor("w_l3", [2, G, 128, GC, HF], f8e4, kind="ExternalInput")
    w4_d = nc.dram_tensor("w_l4", [2, G, 128, GC, HF], f8e4, kind="ExternalInput")
    wout_d = nc.dram_tensor("w_out", [128, 8 * OUT], f16, kind="ExternalInput")
    bias_d = nc.dram_tensor("bias", [1, 3 * SH], f16, kind="ExternalInput")
    id_d = nc.dram_tensor("ident", [8, 8], f16, kind="ExternalInput")
    out_d = nc.dram_tensor("out_partial", [1, OUT], f32, kind="ExternalOutput")

    with tile.TileContext(nc) as tc:
        with (
            tc.tile_pool(name="const", bufs=1) as cp,
            tc.tile_pool(name="wpool", bufs=WBUFS) as wp,
            tc.tile_pool(name="apool", bufs=2) as ap,
            tc.tile_pool(name="pspool", bufs=2, space="PSUM") as pp,
            tc.tile_pool(name="dpool", bufs=2, space="DRAM") as dp,
        ):
            # Dummy collectives on the REAL cc buffer pairs (both
            # channels): absorbs the one-time ncfw rendezvous AND the
            # per-channel mesh algo setup concurrently with layer-1
            # compute + weight prefetch. Content is garbage; the real
            # AllGathers fully overwrite the outputs later.
            cc_in_t = [None, None]
            cc_out_t = [None, None]
            for h in range(2):
                cc_in_t[h] = dp.tile(
                    [1, HF], f16, tag=f"ccin{h}", bufs=1, name="cc_in"
                )
                cc_out_t[h] = dp.tile(
                    [8, HF], f16, tag=f"ccout{h}", bufs=1, name="cc_out"
                )
            nc.gpsimd.collective_compute(
                "AllGather",
                mybir.AluOpType.bypass,
                replica_groups=RG,
                ins=[cc_in_t[0].opt()],
                outs=[cc_out_t[0].opt()],
            )

            one_sb = cp.tile([1, 1], f16)
            nc.gpsimd.memset(one_sb[:], 1.0)

            x_sb = cp.tile([DA, 1], f16)
            nc.scalar.dma_start(x_sb[:], x_d[:])
            win_sb = cp.tile([DA, H], f16)
            nc.scalar.dma_start(win_sb[:], win_d[:])
            bias_sb = cp.tile([1, 3 * SH], f16)
            nc.scalar.dma_start(bias_sb[:], bias_d[:])
            wout_sb = cp.tile([128, 8 * OUT], f16)
            nc.scalar.dma_start(wout_sb[:], wout_d[:])
            ident_sb = cp.tile([8, 8], f16)
            nc.scalar.dma_start(ident_sb[:], id_d[:])

            # ---- Layer 1, replicated: straight into the [128, 2, 32]
            # layout (a8_sb[p, i, c] = a1[(i*32+c)*128 + p]); the (2, 32)
            # split gives DoubleRow lhsT pairs a 32 B pair stride. ----
            a8_sb = ap.tile([128, 2, KC // 2], f8e4, tag="a8")
            for j8 in range(8):
                hi, c0 = j8 // 4, (8 * j8) % 32
                ps1 = pp.tile([128, 1, 8], f32, tag="psL1", bufs=1)
                for jj in range(8):
                    j = 8 * j8 + jj
                    nc.tensor.matmul(
                        ps1[:, 0:1, jj : jj + 1],
                        win_sb[:, 128 * j : 128 * j + 128],
                        x_sb[:],
                        start=True,
                        stop=True,
                    )
                nc.scalar.activation(
                    a8_sb[:, hi : hi + 1, c0 : c0 + 8], ps1[:], SIG
                )

            # ---- helpers ----
            def emit_gather(act_h, hf):
                """AllGather one 512-col output half; returns cc_out."""
                cc_in, cc_out = cc_in_t[hf], cc_out_t[hf]
                nc.scalar.dma_start(cc_in[:], act_h[:])
                nc.gpsimd.collective_compute(
                    "AllGather",
                    mybir.AluOpType.bypass,
                    replica_groups=RG,
                    ins=[cc_in.opt()],
                    outs=[cc_out.opt()],
                )
                return cc_out

            def emit_unpack(cc_out, a_dst, hf_in, dr):
                """[8, 512] gathered half -> 4 PE transposes -> a_dst.
                Column q=8j+r of the transpose holds rank r's cols
                [128j, 128j+128) of this half."""
                g8 = ap.tile([8, HF], f16, tag="g8", name="g8")
                nc.scalar.dma_start(g8[:], cc_out[:])
                if dr:
                    psT = pp.tile([128, 2, 16], f16, tag="psT3", bufs=1)
                    for j in range(4):
                        nc.tensor.matmul(
                            psT[:, j // 2 : j // 2 + 1, 8 * (j % 2) : 8 * (j % 2) + 8],
                            g8[:, 128 * j : 128 * j + 128],
                            ident_sb[:],
                            is_transpose=True,
                            start=True,
                            stop=True,
                        )
                    for i in range(2):
                        nc.vector.tensor_copy(
                            a_dst[:, i : i + 1, 16 * hf_in : 16 * hf_in + 16],
                            psT[:, i : i + 1, :],
                        )
                else:
                    psT = pp.tile([128, 32], f16, tag="psT4", bufs=1)
                    for j in range(4):
                        nc.tensor.matmul(
                            psT[:, 8 * j : 8 * j + 8],
                            g8[:, 128 * j : 128 * j + 128],
                            ident_sb[:],
                            is_transpose=True,
                            start=True,
                            stop=True,
                        )
                    nc.vector.tensor_copy(
                        a_dst[:, 32 * hf_in : 32 * hf_in + 32], psT[:]
                    )

            def emit_hidden(
                w_d, pm, descale, a_in, bias_off, inject_b=None, after_half=None
            ):
                """One hidden layer: 2 output halves x 4 weight groups.
                inject_b() is called before group 2 of half 0 — the spot
                where the previous boundary's B-half unpack goes (its
                AG has landed by then; groups 0-1 touch only A data).
                after_half(hf, act) runs right after each half's sigmoid
                so gather DMAs don't queue behind the other half."""
                outs = []
                ps = [
                    pp.tile([1, HF], f32, tag="psH", bufs=2, name="ps"),
                    pp.tile([1, HF], f32, tag="psH", bufs=2, name="ps"),
                ]

                def emit_group(hf, g):
                    wt = wp.tile(
                        [128, GC, HF],
                        f8e4 if pm is DR else f8e3,
                        tag="w",
                        name="wt",
                    )
                    nc.sync.dma_start(wt[:], w_d[hf, g])
                    if pm is DR:
                        for c in range(GC // 2):
                            k = g * GC + 2 * c
                            nc.tensor.matmul(
                                ps[hf][:],
                                a_in[:, :, k // 2 : k // 2 + 1],
                                wt[:, 2 * c : 2 * c + 2, :],
                                start=(k == 0),
                                stop=False,
                                perf_mode=DR,
                            )
                    else:
                        for c in range(GC):
                            k = g * GC + c
                            nc.tensor.matmul(
                                ps[hf][:],
                                a_in[:, k : k + 1],
                                wt[:, c : c + 1, :],
                                start=(k == 0),
                                stop=False,
                            )

                def finish_half(hf):
                    nc.tensor.matmul(
                        ps[hf][:],
                        one_sb[:],
                        bias_sb[:, bias_off + hf * HF : bias_off + hf * HF + HF],
                        start=False,
                        stop=True,
                    )
                    act_h = ap.tile([1, HF], f16, tag=f"act{hf}", name="act_h")
                    nc.scalar.activation(act_h[:], ps[hf][:], SIG, scale=descale)
                    if after_half is not None:
                        after_half(hf, act_h)
                    outs.append(act_h)

                # A-input groups for BOTH output halves first: the window
                # hiding the previous boundary's B-half AllGather is then
                # the whole A-contraction (half the layer), not a quarter.
                for hf in range(2):
                    for g in range(G // 2):
                        emit_group(hf, g)
                if inject_b is not None:
                    inject_b()
                for hf in range(2):
                    for g in range(G // 2, G):
                        emit_group(hf, g)
                    finish_half(hf)
                return outs

            # ---- Layer 2 (DoubleRow e4m3) ----
            ccs = [None, None]

            def gather_cb(hf, a):
                ccs[hf] = emit_gather(a, hf)

            emit_hidden(w2_d, DR, 1.0 / S_DR, a8_sb, 0, after_half=gather_cb)
            cc_a, cc_b = ccs

            # ---- Layer 3 (DoubleRow e4m3): input a3 [128, 2, 32] fp8 ----
            a3_sb = ap.tile([128, 2, KC // 2], f8e4, tag="a3")
            emit_unpack(cc_a, a3_sb, 0, dr=True)
            ccb = cc_b

            def inject3(cc=ccb):
                emit_unpack(cc, a3_sb, 1, dr=True)

            emit_hidden(
                w3_d, DR, 1.0 / S_DR, a3_sb, SH,
                inject_b=inject3, after_half=gather_cb,
            )
            cc_a, cc_b = ccs

            # ---- Layer 4 (DoubleRow e4m3): input a4 [128, 2, 32] fp8 ----
            a4_sb = ap.tile([128, 2, KC // 2], f8e4, tag="a4")
            emit_unpack(cc_a, a4_sb, 0, dr=True)
            ccb2 = cc_b

            def inject4(cc=ccb2):
                emit_unpack(cc, a4_sb, 1, dr=True)

            act = emit_hidden(w4_d, DR, 1.0 / S_DR, a4_sb, 2 * SH, inject_b=inject4)

            # ---- Output layer: transpose acts to [128, 8], row-sharded
            # partial [8] per core (a2_sb[p, t] = act_local[128t + p]) ----
            psOT = pp.tile([128, 8, 2], f16, tag="psOT", bufs=1)
            for t in range(8):
                hf, off = t // 4, 128 * (t % 4)
                nc.tensor.matmul(
                    psOT[:, t : t + 1, 0:1],
                    act[hf][:, off : off + 128],
                    one_sb[:],
                    is_transpose=True,
                    start=True,
                    stop=True,
                )
            a2_sb = ap.tile([128, 8, 2], f16, tag="a2")
            nc.vector.tensor_copy(a2_sb[:], psOT[:])
            pso = pp.tile([1, OUT], f32, tag="psO", bufs=1)
            for t in range(8):
                nc.tensor.matmul(
                    pso[:],
                    a2_sb[:, t : t + 1, 0:1],
                    wout_sb[:, t * OUT : (t + 1) * OUT],
                    start=(t == 0),
                    stop=(t == 7),
                )
            res_sb = ap.tile([1, OUT], f32, tag="res")
            nc.vector.tensor_copy(res_sb[:], pso[:])
            nc.scalar.dma_start(out_d[:], res_sb[:])

    nc.compile()
    return nc


def _pack_layer(wcol_q, perm):
    """[8192, 1024] quantized core shard -> [2 halves, G, 128, GC, HF],
    rows permuted so weight block b, partition p holds row perm[b, p]."""
    wperm = wcol_q[perm]  # [KC, 128, 1024]
    grp = wperm.reshape(G, GC, 128, 2 * HF).transpose(0, 2, 1, 3)  # [G,128,GC,1024]
    return np.stack([grp[..., :HF], grp[..., HF:]])  # [2, G, 128, GC, HF]


def _prep_inputs(x, s, W_in, W_hh, W_out, b):
    """Shard + quantize + lay out the inputs for each of the 8 cores."""
    import ml_dtypes

    f16 = np.float16
    e4 = ml_dtypes.float8_e4m3
    e3 = ml_dtypes.float8_e3m4

    x_aug = np.concatenate(
        [np.asarray(x), np.asarray(s), np.ones(1, np.float32)]
    ).astype(f16)
    x_aug = np.ascontiguousarray(x_aug.reshape(DA, 1))
    b32 = np.asarray(b, np.float32)  # [5, 8192] (b[4] unused)
    win_aug = np.ascontiguousarray(
        np.concatenate([np.asarray(W_in), b32[0:1]], axis=0).astype(f16)
    )  # [11, 8192]
    Whh = np.asarray(W_hh, np.float32)  # [3, 8192, 8192]
    Wout32 = np.asarray(W_out, np.float32)  # [8192, 8]

    # weight block b (or chunk k), partition p -> global activation row.
    bb = np.arange(KC)[:, None]
    p = np.arange(128)[None, :]
    # L2 (DR): block b pairs with a8 col q=(b%2)*32+b//2 = rows q*128+p.
    perm_l2 = (((bb % 2) * 32 + bb // 2) * 128) + p
    # L3 (DR): pair c=b//2 (c<16: A half), i=b%2; in-half chunk
    # q=16i+(c%16); row = (q%8)*1024 + (c//16)*512 + (q//8)*128 + p.
    c_, i_ = bb // 2, bb % 2
    q_ = 16 * i_ + (c_ % 16)
    perm_l3 = (q_ % 8) * 1024 + (c_ // 16) * 512 + (q_ // 8) * 128 + p
    perm_l4 = perm_l3  # L4 (DR): same unpack scheme as L3

    bias_rows = np.concatenate(
        [b32[1] * S_DR, b32[2] * S_DR, b32[3] * S_DR]
    ).astype(f16)  # [3*8192], host-scaled (zeros in this problem)

    ident = np.eye(8, dtype=f16)

    in_maps = []
    for c in range(NCORES):
        cs, ce = c * SH, (c + 1) * SH
        w2 = _pack_layer((Whh[0][:, cs:ce] * S_DR).astype(e4), perm_l2)
        w3 = _pack_layer((Whh[1][:, cs:ce] * S_DR).astype(e4), perm_l3)
        w4 = _pack_layer((Whh[2][:, cs:ce] * S_DR).astype(e4), perm_l4)
        bias_c = np.concatenate(
            [bias_rows[li * H + cs : li * H + ce] for li in range(3)]
        ).reshape(1, 3 * SH)
        # out layer: a2_sb[p, t] = act_local[128t + p]
        wout_c = np.ascontiguousarray(
            Wout32[cs:ce].reshape(8, 128, OUT).transpose(1, 0, 2)
            .reshape(128, 8 * OUT).astype(f16)
        )
        in_maps.append(
            {
                "x_aug": x_aug,
                "w_in": win_aug,
                "w_l2": np.ascontiguousarray(w2),
                "w_l3": np.ascontiguousarray(w3),
                "w_l4": np.ascontiguousarray(w4),
                "w_out": wout_c,
                "bias": np.ascontiguousarray(bias_c),
                "ident": ident,
            }
        )
    return in_maps


def kernel(**inputs):
    global LAST_RESULTS
    import os

    from concourse import bass_utils

    if "nc" not in _CACHE:
        _CACHE["nc"] = _build_nc()
    nc = _CACHE["nc"]

    in_maps = _prep_inputs(**inputs)
    trace = bool(int(os.environ.get("BASS_TRACE_KERNEL", "0")))
    res = bass_utils.run_bass_kernel_spmd(
        nc, in_maps, core_ids=list(range(NCORES)), trace=trace
    )
    LAST_RESULTS = res
    partials = np.stack([r["out_partial"][0] for r in res.results])  # [8, 8]
    return partials.sum(axis=0).astype(np.float32)
